# revision 1
# baseline (speedup 1.0000x reference)
"""DifferentialMultiHeadAttention TRN2 Bass kernel.

Sharding: 2 branches x 16 heads = 32 head-instances, 4 per core (core 0-3:
branch 1, core 4-7: branch 2). Each core computes its heads' attention,
applies its lambda-scaled head-output projection and the full final proj on
its rank-partial y; the host sums the 8 partial outputs (valid because wo,
the lambda-mix and proj are linear) and adds the folded bias vector.

QKV biases are handled exactly by augmenting the contraction dim: x' =
[x, 1, 0..] (D 1024 -> 1152 = 9*128), w' = [w; b; 0..]. Matmuls run in bf16
(fp32 PSUM accumulation); MM_DTYPE="f32r" switches to TF32-like float32r
(~15x better accuracy, ~1.9x slower: no fast-weight-load for 4-byte types).
The final projection proj_w is folded into the per-head output projections
on the host (z = sum_h out_h @ (lamf * wo_h @ proj_w)), eliminating the
whole proj stage on device; each core emits a transposed partial zT.

Softmax is computed without max-subtraction (scores are O(5), exp is safe in
fp32) via a transposed layout: scoresT[s,t] tiles feed exp (ScalarE,
PSUM->SBUF), then AV accumulates lhsT=[v|1] so PSUM row 64 is the softmax
denominator; the reciprocal row is broadcast across partitions with a K=1
outer-product matmul and applied with one vector multiply.
"""

import sys

for _p in ("/opt/trn_rl_repo", "/opt/pypackages"):
    if _p not in sys.path:
        sys.path.append(_p)

import numpy as np
import ml_dtypes


MM_DTYPE = "bf16"   # "bf16" | "f32r"  (matmul operand precision)
MM_NP = ml_dtypes.bfloat16 if MM_DTYPE == "bf16" else np.float32

DIM, H, HD = 1024, 16, 64
B = 2
DA = 1152          # augmented contraction dim (bias row + pad)
NDT = DA // 128    # 9 d-tiles
NPT = DIM // 128   # 8 d-tiles for proj
CH = 512           # token chunk size
NH = 4             # heads per core
NCORES = 8


def build(S=2048):
    """Build the per-core SPMD Bass program for per-batch seq len S."""
    import concourse.bacc as bacc
    import concourse.bass as bass
    import concourse.mybir as mybir
    import concourse.tile as tile

    f32 = mybir.dt.float32
    f32r = {"bf16": mybir.dt.bfloat16, "f32r": mybir.dt.float32r}[MM_DTYPE]

    T = B * S                    # total tokens
    NC = S // CH                 # chunks per batch
    NST = S // 128               # s-tiles per batch

    nc = bacc.Bacc("TRN2", target_bir_lowering=False, debug=False,
                   num_devices=NCORES)

    xta = nc.dram_tensor("xta", [DA, T], f32r, kind="ExternalInput")
    wq = nc.dram_tensor("wq", [DA, 256], f32r, kind="ExternalInput")
    wk = nc.dram_tensor("wk", [DA, 256], f32r, kind="ExternalInput")
    wv = nc.dram_tensor("wv", [DA, 256], f32r, kind="ExternalInput")
    wo = nc.dram_tensor("wo", [256, DIM], f32r, kind="ExternalInput")
    one = nc.dram_tensor("one", [128, 64], f32, kind="ExternalInput")
    onem = nc.dram_tensor("onem", [128, 64], f32r, kind="ExternalInput")
    z = nc.dram_tensor("z", [DIM, T], f32, kind="ExternalOutput")

    with tile.TileContext(nc) as tc:
        with (
            nc.allow_low_precision(
                reason="f32r storage is 4-byte fp32; PSUM accumulation stays fp32"),
            tc.tile_pool(name="consts", bufs=1) as consts,
            tc.tile_pool(name="kv", bufs=1) as kv,
            tc.tile_pool(name="xp", bufs=2) as xp,
            tc.tile_pool(name="work", bufs=4) as work,
            tc.tile_pool(name="outp", bufs=2) as outp,
            tc.tile_pool(name="scp", bufs=2, space="PSUM") as scp,
            tc.tile_pool(name="flx", bufs=4, space="PSUM") as flx,
        ):
            wq_sb = consts.tile([128, NDT, 256], f32r)
            wk_sb = consts.tile([128, NDT, 256], f32r)
            wv_sb = consts.tile([128, NDT, 256], f32r)
            wo_sb = consts.tile([128, 2, DIM], f32r)
            ones_sb = consts.tile([1, 64], f32)
            ones_mm = consts.tile([1, 64], f32r)

            nc.gpsimd.dma_start(out=wq_sb, in_=wq.ap().rearrange("(dt p) m -> p dt m", p=128))
            nc.gpsimd.dma_start(out=wk_sb, in_=wk.ap().rearrange("(dt p) m -> p dt m", p=128))
            nc.gpsimd.dma_start(out=wv_sb, in_=wv.ap().rearrange("(dt p) m -> p dt m", p=128))
            nc.gpsimd.dma_start(out=ones_sb, in_=one.ap()[0:1, 0:64])
            nc.gpsimd.dma_start(out=ones_mm, in_=onem.ap()[0:1, 0:64])
            # wo' (= lamf * wo @ proj_w, folded host-side) is first used in
            # phase B; keep it off the startup critical path
            nc.gpsimd.dma_start(out=wo_sb, in_=wo.ap().rearrange("(pk p) n -> p pk n", p=128))

            xre = xta.ap().rearrange("(dt p) t -> p dt t", p=128)

            wu = consts.tile([128, CH], f32r, name="wu")
            nc.vector.memset(wu, 0.25)
            for wi in range(96):
                wp = flx.tile([128, CH], f32, tag="flex", name=f"wp{wi}")
                nc.tensor.matmul(wp[:], wu[:, 0:128], wu[:], start=True, stop=True)

            opq = []

            def drain(n):
                for _ in range(min(n, len(opq))):
                    opq.pop(0)()

            def queue_zt(tb, outT):
                # head-output projection (wo' = lamf * wo @ proj_w folded on
                # the host) of a finished chunk, split into closures that the
                # attention loop of the NEXT chunk drains one at a time to
                # fill PE bubbles left by the exp dependency chain. Output is
                # zT [DIM, T]; the host sums and transposes.
                def zt_op(eo):
                    def f():
                        zp = flx.tile([128, CH], f32, tag="flex",
                                      name=f"zp{tb}_{eo}")
                        for pk in range(2):
                            nc.tensor.matmul(
                                zp[:], (wo_sb[:, pk, eo * 128:(eo + 1) * 128]),
                                (outT[:, pk, :]),
                                start=(pk == 0), stop=(pk == 1))
                        zs = work.tile([128, CH], f32, tag="zs",
                                       name=f"zs{tb}_{eo}")
                        nc.vector.tensor_copy(zs[:], zp[:])
                        nc.sync.dma_start(
                            out=z.ap()[eo * 128:(eo + 1) * 128, tb:tb + CH],
                            in_=zs[:])
                    return f

                for eo in range(NPT):
                    opq.append(zt_op(eo))

            for b in range(B):
                kT = kv.tile([128, 2, S], f32r, tag="kT")
                qT = kv.tile([128, 2, S], f32r, tag="qT")
                va = kv.tile([128, NST, NH, 65], f32r, tag="va")
                nc.sync.dma_start(
                    out=va[:, :, :, 64:65],
                    in_=onem.ap()[:, 0:NST * NH].rearrange(
                        "p (st h) -> p st h", st=NST))

                # ---- phase A: Q/K/V generation for this batch ----
                for c in range(NC):
                    tb = b * S + c * CH
                    x_blk = xp.tile([128, NDT, CH], f32r, tag="x")
                    nc.sync.dma_start(out=x_blk, in_=xre[:, :, tb:tb + CH])

                    for wsb, dst in ((wq_sb, qT), (wk_sb, kT)):
                        for pk in range(2):
                            ps = flx.tile([128, CH], f32, tag="flex")
                            for dt_i in range(NDT):
                                nc.tensor.matmul(
                                    ps[:], (wsb[:, dt_i, 128 * pk:128 * pk + 128]),
                                    (x_blk[:, dt_i, :]),
                                    start=(dt_i == 0), stop=(dt_i == NDT - 1))
                            nc.vector.tensor_copy(
                                dst[:, pk, c * CH:(c + 1) * CH], ps[:])

                    for tt in range(4):
                        ps = flx.tile([128, 256], f32, tag="flex")
                        for dt_i in range(NDT):
                            nc.tensor.matmul(
                                ps[:], (x_blk[:, dt_i, 128 * tt:128 * tt + 128]),
                                (wv_sb[:, dt_i, :]),
                                start=(dt_i == 0), stop=(dt_i == NDT - 1))
                        st = c * 4 + tt
                        nc.vector.tensor_copy(
                            va[:, st, :, 0:64],
                            ps.rearrange("p (h d) -> p h d", h=NH))

                # ---- phase B: attention per chunk; wo+proj pipelined one
                # chunk behind so the PE never stalls on the normalization
                # tail or at chunk/batch boundaries ----
                for c in range(NC):
                    tb = b * S + c * CH
                    outT = outp.tile([128, 2, CH], f32r, tag="outT",
                                     name=f"outT{b}_{c}")

                    for pk in range(2):
                        # head pair (2*pk, 2*pk+1): score matmuls interleave
                        # rows 0-63 / 64-127 so they run concurrently on the
                        # PE's disjoint row-groups.
                        avs = [flx.tile([128, CH], f32, tag="flex",
                                        name=f"av{pk}_{i}")
                               for i in range(2)]
                        for sp in range(NST // 2):
                            scs = [scp.tile([128, 2, CH], f32, tag="sc",
                                            name=f"sc{pk}_{sp}_{i}")
                                   for i in range(2)]
                            for j in range(2):
                                st = 2 * sp + j
                                for hh in range(2):
                                    row = 64 * hh
                                    nc.tensor.matmul(
                                        scs[hh][:, j, :],
                                        (kT[row:row + 64, pk, st * 128:(st + 1) * 128]),
                                        (qT[row:row + 64, pk, c * CH:(c + 1) * CH]),
                                        start=True, stop=True)
                            exs = []
                            for hh in range(2):
                                ex = work.tile([128, 2, CH], f32r, tag="ex")
                                nc.scalar.activation(
                                    ex[:], scs[hh][:],
                                    mybir.ActivationFunctionType.Exp)
                                exs.append(ex)
                            for j in range(2):
                                st = 2 * sp + j
                                for hh in range(2):
                                    h = 2 * pk + hh
                                    nc.tensor.matmul(
                                        avs[hh][0:65, :], (va[:, st, h, :]),
                                        (exs[hh][:, j, :]),
                                        start=(st == 0), stop=(st == NST - 1))
                            if sp % 2 == 1:
                                drain(1)

                        dens, rcpms, us = [], [], []
                        for hh in range(2):
                            # den on ScalarE and u on VectorE in parallel so
                            # both av PSUM slots free as fast as possible
                            av = avs[hh]
                            den = work.tile([1, CH], f32, tag="den",
                                            name=f"den{pk}_{hh}")
                            nc.vector.tensor_copy(den[:], av[64:65, :])
                            u = work.tile([64, CH], f32r, tag="u", bufs=6,
                                          name=f"u{pk}_{hh}")
                            nc.vector.tensor_copy(u[:], av[0:64, :])
                            dens.append(den); us.append(u)
                        rcpms = []
                        for hh in range(2):
                            rcp = work.tile([1, CH], f32, tag="rcp",
                                            name=f"rcp{pk}_{hh}")
                            nc.vector.reciprocal_approx_fast(rcp[:], dens[hh][:])
                            rcpm = work.tile([1, CH], f32r, tag="rcpm",
                                             bufs=6, name=f"rcpm{pk}_{hh}")
                            nc.vector.tensor_copy(rcpm[:], rcp[:])
                            rcpms.append(rcpm)
                        drain(2)
                        for hh in range(2):
                            bc = flx.tile([64, CH], f32, tag="flex",
                                          name=f"bc{pk}_{hh}")
                            nc.tensor.matmul(bc[:], (ones_mm[:]), (rcpms[hh][:]),
                                             start=True, stop=True)
                            nc.vector.tensor_mul(
                                outT[64 * hh:64 * hh + 64, pk, :],
                                us[hh][:], bc[:])

                    queue_zt(tb, outT)

            drain(len(opq))

    nc.compile()
    return nc


def get_lambda(lambda_param, layer_idx):
    lf = np.clip(float(np.asarray(layer_idx)) * 0.3, 0.0, 5.0)
    offset = 0.6 * np.exp(-lf)
    lam = (1.0 / (1.0 + np.exp(-float(np.asarray(lambda_param).reshape(-1)[0])))
           ) * (1.0 - offset) + 0.2
    return float(np.clip(lam, 0.1, 0.9))


def prep(inputs, S=2048):
    """Host-side shard prep: returns (in_maps, bias_vec)."""
    x = np.asarray(inputs["x"], np.float32)
    T = B * S
    x2 = np.ascontiguousarray(x.reshape(T, DIM))
    xta = np.zeros((DA, T), np.float32)
    xta[:DIM] = x2.T
    xta[DIM] = 1.0

    lam = get_lambda(inputs["lambda_param"], inputs["layer_idx"])
    pw = np.asarray(inputs["proj_w"], np.float32)
    xta_mm = xta.astype(MM_NP)

    in_maps = []
    for c in range(NCORES):
        br = c // 4 + 1
        lamf = (1.0 - lam) if br == 1 else lam
        hs = slice(4 * (c % 4), 4 * (c % 4) + 4)

        def aug(w, bias, scale=1.0):
            wa = np.zeros((DA, NH, HD), np.float32)
            wa[:DIM] = np.asarray(w, np.float32)[:, hs]
            wa[DIM] = np.asarray(bias, np.float32)[hs]
            return np.ascontiguousarray(
                (wa * scale).reshape(DA, NH * HD)).astype(MM_NP)

        wo_c = np.ascontiguousarray(
            ((np.asarray(inputs[f"wo{br}"], np.float32)[hs] * lamf
              ).reshape(256, DIM) @ pw).astype(MM_NP))
        in_maps.append({
            "one": np.ones((128, 64), np.float32),
            "onem": np.ones((128, 64), MM_NP),
            "xta": xta_mm,
            "wq": aug(inputs[f"wq{br}"], inputs[f"bq{br}"], 1.0 / np.sqrt(HD)),
            "wk": aug(inputs[f"wk{br}"], inputs[f"bk{br}"]),
            "wv": aug(inputs[f"wv{br}"], inputs[f"bv{br}"]),
            "wo": wo_c,
        })

    lam32 = np.float32(lam)
    yb = ((1 - lam32) * np.asarray(inputs["bo1"], np.float32)
          + lam32 * np.asarray(inputs["bo2"], np.float32))
    bias_vec = yb.astype(np.float64) @ pw.astype(np.float64) \
        + np.asarray(inputs["proj_b"], np.float64)
    return in_maps, bias_vec


_NC_CACHE = {}


def _get_nc(S=2048):
    if S not in _NC_CACHE:
        _NC_CACHE[S] = build(S)
    return _NC_CACHE[S]


def run(inputs, S=2048, trace=False):
    """Returns (full_output, exec_time_ns_or_None)."""
    from concourse import bass_utils

    nc = _get_nc(S)
    in_maps, bias_vec = prep(inputs, S)
    res = bass_utils.run_bass_kernel_spmd(
        nc, in_maps, core_ids=list(range(NCORES)), trace=trace)
    accT = np.zeros((DIM, B * S), np.float64)
    for c in range(NCORES):
        accT += res.results[c]["z"].astype(np.float64)
    out = (accT.T + bias_vec).reshape(B, S, DIM).astype(np.float32)
    return out, res.exec_time_ns


def kernel(**inputs):
    out, _ = run(inputs, S=2048, trace=False)
    return out



# revision 6
# speedup vs baseline: 1.3166x; 1.3166x over previous
"""DifferentialMultiHeadAttention TRN2 Bass kernel.

Sharding: 2 branches x 16 heads = 32 head-instances, 4 per core (core 0-3:
branch 1, core 4-7: branch 2). Each core computes its heads' attention,
applies its lambda-scaled head-output projection (with the final proj folded
in host-side); the host sums the 8 partial outputs and adds the folded bias.

Performance architecture: the TRN2 PE clock ramps with *continuous* execution
and drops on any idle gap, so the whole program is built as one gap-free PE
instruction stream. Attention blocks (one (chunk, head-pair) at a time) are
software-pipelined: scores for step sp issue ahead of the AV accumulation of
step sp-1, and every exp-latency bubble is plugged by popping a closure from
a fill queue holding the *next* phase's work (QKV projection chains of the
other batch, deferred wo-projections of finished chunks, dummy matmuls as a
last resort).

QKV biases: bq/bk are added to qT/kT during the PSUM->SBUF copy as a
per-partition scalar add (DVE tensor_scalar); bv is folded into the host-side
output bias (softmax rows sum to 1, so out_h = AV/den + bv exactly).
Softmax runs without max-subtraction in a transposed layout (scoresT[s,t]);
the denominator comes from a ones-column appended to V; the reciprocal pair
for both heads is broadcast across partitions with a single K=2 matmul
against a block-indicator matrix and applied with one vector multiply.
Matmuls run in bf16 with fp32 PSUM accumulation.
"""

import sys

for _p in ("/opt/trn_rl_repo", "/opt/pypackages"):
    if _p not in sys.path:
        sys.path.append(_p)

import numpy as np
import ml_dtypes

MM_NP = ml_dtypes.bfloat16

DIM, H, HD = 1024, 16, 64
B = 2
NDT = DIM // 128   # 8 d-tiles
CH = 512           # token chunk size
NH = 4             # heads per core
NCORES = 8
NWARM = 28         # startup dummy matmuls (cover x DMA + clock ramp)


def build(S=2048):
    """Build the per-core SPMD Bass program for per-batch seq len S."""
    import concourse.bacc as bacc
    import concourse.bass as bass
    import concourse.mybir as mybir
    import concourse.tile as tile

    f32 = mybir.dt.float32
    bf16 = mybir.dt.bfloat16

    T = B * S
    NC = S // CH                 # chunks per batch
    NST = S // 128               # s-tiles per batch

    nc = bacc.Bacc("TRN2", target_bir_lowering=False, debug=False,
                   num_devices=NCORES)

    xt = nc.dram_tensor("xt", [DIM, T], bf16, kind="ExternalInput")
    wq = nc.dram_tensor("wq", [DIM, 256], bf16, kind="ExternalInput")
    wk = nc.dram_tensor("wk", [DIM, 256], bf16, kind="ExternalInput")
    wv = nc.dram_tensor("wv", [DIM, 256], bf16, kind="ExternalInput")
    wo = nc.dram_tensor("wo", [256, DIM], bf16, kind="ExternalInput")
    qkb = nc.dram_tensor("qkb", [128, 4], f32, kind="ExternalInput")
    z = nc.dram_tensor("z", [DIM, T], f32, kind="ExternalOutput")

    with tile.TileContext(nc) as tc:
        with (
            nc.allow_low_precision(
                reason="bf16 operands; PSUM accumulation stays fp32"),
            tc.tile_pool(name="scp", bufs=2, space="PSUM") as scp,
            tc.tile_pool(name="avp", bufs=2, space="PSUM") as avp,
            tc.tile_pool(name="flxA", bufs=1, space="PSUM") as flxA,
            tc.tile_pool(name="flxB", bufs=1, space="PSUM") as flxB,
            tc.tile_pool(name="consts", bufs=1) as consts,
            tc.tile_pool(name="kv", bufs=2) as kv,
            tc.tile_pool(name="xp", bufs=2 * NC) as xp,
            tc.tile_pool(name="work", bufs=4) as work,
            tc.tile_pool(name="outp", bufs=2 * NC) as outp,
        ):
            wq_sb = consts.tile([128, NDT, 256], bf16)
            wk_sb = consts.tile([128, NDT, 256], bf16)
            wv_sb = consts.tile([128, NDT, 256], bf16)
            wo_sb = consts.tile([128, 2, DIM], bf16)
            qkb_sb = consts.tile([128, 4], f32)
            ones_mm = consts.tile([1, 64], bf16)
            wu = consts.tile([128, CH], bf16, name="wu")

            # weights first (QK-gen is the first real work), x afterwards
            nc.gpsimd.dma_start(out=wq_sb, in_=wq.ap().rearrange("(dt p) m -> p dt m", p=128))
            nc.gpsimd.dma_start(out=wk_sb, in_=wk.ap().rearrange("(dt p) m -> p dt m", p=128))
            nc.gpsimd.dma_start(out=wv_sb, in_=wv.ap().rearrange("(dt p) m -> p dt m", p=128))
            nc.gpsimd.dma_start(out=qkb_sb, in_=qkb.ap())
            nc.gpsimd.dma_start(out=wo_sb, in_=wo.ap().rearrange("(pk p) n -> p pk n", p=128))

            xre = xt.ap().rearrange("(dt p) t -> p dt t", p=128)
            x_blks = {}
            for b in range(B):
                for c in range(NC):
                    tb = b * S + c * CH
                    xb = xp.tile([128, NDT, CH], bf16, tag="x", name=f"x{b}_{c}")
                    nc.sync.dma_start(out=xb, in_=xre[:, :, tb:tb + CH])
                    x_blks[(b, c)] = xb

            nc.vector.memset(wu, 0.25)
            nc.vector.memset(ones_mm, 1.0)

            kT, qT, va = {}, {}, {}
            for b in range(B):
                kT[b] = kv.tile([128, 2, S], bf16, tag="kT", name=f"kT{b}")
                qT[b] = kv.tile([128, 2, S], bf16, tag="qT", name=f"qT{b}")
                va[b] = kv.tile([128, NST, NH, 65], bf16, tag="va", name=f"va{b}")
                nc.vector.memset(va[b][:, :, :, 64:65], 1.0)

            # ---- startup warmup: ramp the PE clock while DMAs land ----
            for wi in range(NWARM):
                wp = flxB.tile([128, CH], f32, tag="fill", name=f"warm{wi}")
                nc.tensor.matmul(wp[:], wu[:, 0:128], wu[:], start=True, stop=True)

            # ---- fill queues + drain ----
            opq_m = []   # mandatory (QKV chains) — drained first, FIFO
            opq_f = []   # deferred (bc / wo-projection) — drained when m dry
            ndummy = [0]

            def dummy_op():
                wp = flxB.tile([128, CH], f32, tag="fill",
                               name=f"dum{ndummy[0]}")
                nc.tensor.matmul(wp[:], wu[:, 0:128], wu[:], start=True, stop=True)
                ndummy[0] += 1

            def drain(n):
                for _ in range(n):
                    if opq_m:
                        opq_m.pop(0)()
                    elif opq_f:
                        opq_f.pop(0)()
                    else:
                        dummy_op()

            def drain_all_m():
                while opq_m:
                    opq_m.pop(0)()

            # ---- QKV projection chain builders (list of closures) ----
            def qk_closures(b, c, wsb, dstT, bidx, pk):
                st8 = {}

                def part(d0):
                    def f():
                        if d0 == 0:
                            st8["ps"] = flxA.tile([128, CH], f32, tag="chain",
                                                  name=f"qk{b}_{c}_{bidx}_{pk}")
                        ps = st8["ps"]
                        for dt_i in (d0, d0 + 1):
                            nc.tensor.matmul(
                                ps[:], wsb[:, dt_i, 128 * pk:128 * pk + 128],
                                x_blks[(b, c)][:, dt_i, :],
                                start=(dt_i == 0), stop=(dt_i == NDT - 1))
                        if d0 == NDT - 2:
                            nc.vector.tensor_scalar(
                                dstT[b][:, pk, c * CH:(c + 1) * CH], ps[:],
                                qkb_sb[:, bidx + pk:bidx + pk + 1], None,
                                mybir.AluOpType.add)
                    return f

                return [part(d0) for d0 in range(0, NDT, 2)]

            def v_closures(b, c, tt):
                st8 = {}

                def part(d0):
                    def f():
                        if d0 == 0:
                            st8["ps"] = flxA.tile([128, CH], f32, tag="chain",
                                                  name=f"v{b}_{c}_{tt}")
                        ps = st8["ps"]
                        for dt_i in range(d0, d0 + 4):
                            nc.tensor.matmul(
                                ps[:, 0:256], x_blks[(b, c)][:, dt_i, 128 * tt:128 * tt + 128],
                                wv_sb[:, dt_i, :],
                                start=(dt_i == 0), stop=(dt_i == NDT - 1))
                        if d0 == NDT - 4:
                            st = c * 4 + tt
                            nc.vector.tensor_copy(
                                va[b][:, st, :, 0:64],
                                ps[:, 0:256].rearrange("p (h d) -> p h d", h=NH))
                    return f

                return [part(0), part(4)]

            def run_now(cls):
                for f in cls:
                    f()

            # ---- deferred output projection (wo' = lamf*wo@proj folded) ----
            outTs = {}

            def queue_bc(b, c, pk, u, rcp2m):
                outT = outTs[(b, c)]

                def f():
                    bc = flxB.tile([128, CH], f32, tag="fill",
                                   name=f"bc{b}_{c}_{pk}")
                    for hh in range(2):
                        nc.tensor.matmul(bc[64 * hh:64 * hh + 64, :],
                                         ones_mm[:], rcp2m[:, hh, :],
                                         start=True, stop=True)
                    nc.vector.tensor_mul(outT[:, pk, :], u[:], bc[:])
                opq_f.append(f)

            def queue_zt(b, c):
                tb = b * S + c * CH
                outT = outTs[(b, c)]

                def zt_op(eo):
                    def f():
                        zp = flxB.tile([128, CH], f32, tag="fill",
                                       name=f"zp{b}_{c}_{eo}")
                        for pk in range(2):
                            nc.tensor.matmul(
                                zp[:], wo_sb[:, pk, eo * 128:(eo + 1) * 128],
                                outT[:, pk, :],
                                start=(pk == 0), stop=(pk == 1))
                        zs = work.tile([128, CH], f32, tag="zs",
                                       name=f"zs{b}_{c}_{eo}")
                        nc.vector.tensor_copy(zs[:], zp[:])
                        nc.sync.dma_start(
                            out=z.ap()[eo * 128:(eo + 1) * 128, tb:tb + CH],
                            in_=zs[:])
                    return f

                for eo in range(NDT):
                    opq_f.append(zt_op(eo))

            # ---- attention block: one (batch, chunk, head-pair) ----
            def attn_block(b, c, pk):
                if pk == 0:
                    outTs[(b, c)] = outp.tile([128, 2, CH], bf16, tag="outT",
                                              name=f"outT{b}_{c}")
                avs = [avp.tile([128, CH], f32, tag="av",
                                name=f"av{b}_{c}_{pk}_{hh}") for hh in range(2)]

                def emit_av(hh, ex, sp):
                    h = 2 * pk + hh
                    for j in range(2):
                        st = 2 * sp + j
                        nc.tensor.matmul(
                            avs[hh][0:65, :], va[b][:, st, h, :], ex[:, j, :],
                            start=(st == 0), stop=(st == NST - 1))

                prev_ex = None
                for sp in range(NST // 2):
                    exs = []
                    for hh in range(2):
                        row = 64 * hh
                        sc = scp.tile([128, 2, CH], f32, tag="sc",
                                      name=f"sc{b}_{c}_{pk}_{sp}_{hh}")
                        for j in range(2):
                            st = 2 * sp + j
                            nc.tensor.matmul(
                                sc[:, j, :],
                                kT[b][row:row + 64, pk, st * 128:(st + 1) * 128],
                                qT[b][row:row + 64, pk, c * CH:(c + 1) * CH],
                                start=True, stop=True)
                        ex = work.tile([128, 2, CH], bf16, tag="ex", bufs=4,
                                       name=f"ex{b}_{c}_{pk}_{sp}_{hh}")
                        nc.scalar.activation(
                            ex[:], sc[:], mybir.ActivationFunctionType.Exp)
                        exs.append(ex)
                    drain(1)
                    if prev_ex is not None:
                        emit_av(1, prev_ex[1], sp - 1)
                    else:
                        drain(1)
                    drain(1)
                    emit_av(0, exs[0], sp)
                    prev_ex = exs
                drain(1)
                emit_av(1, prev_ex[1], NST // 2 - 1)

                # normalization epilogue (DVE; bc matmuls deferred to fill queue)
                den2 = work.tile([1, 2, CH], f32, tag="den", bufs=2,
                                 name=f"den{b}_{c}_{pk}")
                nc.vector.tensor_copy(den2[:, 0, :], avs[0][64:65, :])
                nc.vector.tensor_copy(den2[:, 1, :], avs[1][64:65, :])
                rcp2 = work.tile([1, 2, CH], f32, tag="rcp", bufs=2,
                                 name=f"rcp{b}_{c}_{pk}")
                nc.vector.reciprocal_approx_fast(rcp2[:], den2[:])
                rcp2m = work.tile([1, 2, CH], bf16, tag="rcpm", bufs=4,
                                  name=f"rcpm{b}_{c}_{pk}")
                nc.vector.tensor_copy(rcp2m[:], rcp2[:])
                u = work.tile([128, CH], bf16, tag="u", bufs=4,
                              name=f"u{b}_{c}_{pk}")
                nc.vector.tensor_copy(u[0:64, :], avs[0][0:64, :])
                nc.vector.tensor_copy(u[64:128, :], avs[1][0:64, :])
                queue_bc(b, c, pk, u, rcp2m)
                if pk == 1:
                    queue_zt(b, c)

            # ================= program =================
            # phase A(b0) inline & dense: K all chunks, V all chunks, Q(c0)
            for c in range(NC):
                for pk in range(2):
                    run_now(qk_closures(0, c, wk_sb, kT, 2, pk))
            for c in range(NC):
                for tt in range(4):
                    run_now(v_closures(0, c, tt))
            for pk in range(2):
                run_now(qk_closures(0, 0, wq_sb, qT, 0, pk))

            # queue the rest: Q(b0,c1..), then all of batch 1's projections
            for c in range(1, NC):
                for pk in range(2):
                    opq_m += qk_closures(0, c, wq_sb, qT, 0, pk)
            for c in range(NC):
                for pk in range(2):
                    opq_m += qk_closures(1, c, wk_sb, kT, 2, pk)
            for pk in range(2):
                opq_m += qk_closures(1, 0, wq_sb, qT, 0, pk)
            for c in range(NC):
                for tt in range(4):
                    opq_m += v_closures(1, c, tt)
            for c in range(1, NC):
                for pk in range(2):
                    opq_m += qk_closures(1, c, wq_sb, qT, 0, pk)

            # phase B(b0): fill slots consume opq_m (A(b1) work)
            for c in range(NC):
                for pk in range(2):
                    attn_block(0, c, pk)

            # everything batch 1 needs must be emitted before its blocks
            drain_all_m()

            # phase B(b1): fill slots consume opq_f (deferred wo-projections)
            for c in range(NC):
                for pk in range(2):
                    attn_block(1, c, pk)

            while opq_f:
                opq_f.pop(0)()

    nc.compile()
    return nc


def get_lambda(lambda_param, layer_idx):
    lf = np.clip(float(np.asarray(layer_idx)) * 0.3, 0.0, 5.0)
    offset = 0.6 * np.exp(-lf)
    lam = (1.0 / (1.0 + np.exp(-float(np.asarray(lambda_param).reshape(-1)[0])))
           ) * (1.0 - offset) + 0.2
    return float(np.clip(lam, 0.1, 0.9))


def prep(inputs, S=2048):
    """Host-side shard prep: returns (in_maps, bias_vec)."""
    x = np.asarray(inputs["x"], np.float32)
    T = B * S
    x2 = np.ascontiguousarray(x.reshape(T, DIM))
    xt = np.ascontiguousarray(x2.T).astype(MM_NP)

    lam = get_lambda(inputs["lambda_param"], inputs["layer_idx"])
    pw = np.asarray(inputs["proj_w"], np.float32)
    sc_q = 1.0 / np.sqrt(HD)

    in_maps = []
    for core in range(NCORES):
        br = core // 4 + 1
        lamf = (1.0 - lam) if br == 1 else lam
        hs = slice(4 * (core % 4), 4 * (core % 4) + 4)

        def pick(w, scale=1.0):
            wa = np.asarray(w, np.float32)[:, hs] * scale
            return np.ascontiguousarray(wa.reshape(DIM, NH * HD)).astype(MM_NP)

        # per-partition q/k biases: row r of the pk head-pair tile is
        # head (2pk + r//64), dim r%64
        bq = np.asarray(inputs[f"bq{br}"], np.float32)[hs] * sc_q
        bk = np.asarray(inputs[f"bk{br}"], np.float32)[hs]
        qkb = np.zeros((128, 4), np.float32)
        for pk in range(2):
            qkb[:, 0 + pk] = bq[2 * pk:2 * pk + 2].reshape(128)
            qkb[:, 2 + pk] = bk[2 * pk:2 * pk + 2].reshape(128)

        wo_c = np.ascontiguousarray(
            ((np.asarray(inputs[f"wo{br}"], np.float32)[hs] * lamf
              ).reshape(256, DIM) @ pw).astype(MM_NP))
        in_maps.append({
            "xt": xt,
            "wq": pick(inputs[f"wq{br}"], sc_q),
            "wk": pick(inputs[f"wk{br}"]),
            "wv": pick(inputs[f"wv{br}"]),
            "wo": wo_c,
            "qkb": qkb,
        })

    lam64 = np.float64(lam)
    bias_vec = np.zeros((DIM,), np.float64)
    for br, lamf in ((1, 1.0 - lam64), (2, lam64)):
        bo = np.asarray(inputs[f"bo{br}"], np.float64)
        bv = np.asarray(inputs[f"bv{br}"], np.float64).reshape(H * HD)
        wo_full = np.asarray(inputs[f"wo{br}"], np.float64).reshape(H * HD, DIM)
        bias_vec += lamf * (bo + bv @ wo_full)
    bias_vec = bias_vec @ pw.astype(np.float64) \
        + np.asarray(inputs["proj_b"], np.float64)
    return in_maps, bias_vec


_NC_CACHE = {}


def _get_nc(S=2048):
    if S not in _NC_CACHE:
        _NC_CACHE[S] = build(S)
    return _NC_CACHE[S]


def run(inputs, S=2048, trace=False):
    """Returns (full_output, exec_time_ns_or_None)."""
    from concourse import bass_utils

    nc = _get_nc(S)
    in_maps, bias_vec = prep(inputs, S)
    res = bass_utils.run_bass_kernel_spmd(
        nc, in_maps, core_ids=list(range(NCORES)), trace=trace)
    accT = np.zeros((DIM, B * S), np.float64)
    for c in range(NCORES):
        accT += res.results[c]["z"].astype(np.float64)
    out = (accT.T + bias_vec).reshape(B, S, DIM).astype(np.float32)
    return out, res.exec_time_ns


def kernel(**inputs):
    out, _ = run(inputs, S=2048, trace=False)
    return out


# revision 12
# speedup vs baseline: 1.3853x; 1.0522x over previous
"""DifferentialMultiHeadAttention TRN2 Bass kernel.

Sharding: 2 branches x 16 heads = 32 head-instances, 4 per core (core 0-3:
branch 1, core 4-7: branch 2). Each core computes its heads' attention,
applies its lambda-scaled head-output projection (with the final proj folded
in host-side); the host sums the 8 partial outputs and adds the folded bias.

Performance architecture: the TRN2 PE clock ramps with *continuous* execution
and drops on any idle gap, so the whole program is built as one gap-free PE
instruction stream. Attention blocks (one (chunk, head-pair) at a time) are
software-pipelined: scores for step sp issue ahead of the AV accumulation of
step sp-1, and every exp-latency bubble is plugged by popping a closure from
a fill queue holding the *next* phase's work (QKV projection chains of the
other batch, deferred wo-projections of finished chunks, dummy matmuls as a
last resort).

QKV biases: bq/bk are added to qT/kT during the PSUM->SBUF copy as a
per-partition scalar add (DVE tensor_scalar); bv is folded into the host-side
output bias (softmax rows sum to 1, so out_h = AV/den + bv exactly).
Softmax runs without max-subtraction in a transposed layout (scoresT[s,t]);
the denominator comes from a ones-column appended to V; the reciprocal pair
for both heads is broadcast across partitions with a single K=2 matmul
against a block-indicator matrix and applied with one vector multiply.
Matmuls run in bf16 with fp32 PSUM accumulation.
"""

import sys

for _p in ("/opt/trn_rl_repo", "/opt/pypackages"):
    if _p not in sys.path:
        sys.path.append(_p)

import numpy as np
import ml_dtypes

MM_NP = ml_dtypes.bfloat16

DIM, H, HD = 1024, 16, 64
B = 2
NDT = DIM // 128   # 8 d-tiles
CH = 512           # token chunk size
NH = 4             # heads per core
NCORES = 8
NWARM = 38         # startup dummy matmuls (cover x DMA + clock ramp)


def build(S=2048):
    """Build the per-core SPMD Bass program for per-batch seq len S."""
    import concourse.bacc as bacc
    import concourse.bass as bass
    import concourse.mybir as mybir
    import concourse.tile as tile

    f32 = mybir.dt.float32
    bf16 = mybir.dt.bfloat16

    T = B * S
    NC = S // CH                 # chunks per batch
    NST = S // 128               # s-tiles per batch

    nc = bacc.Bacc("TRN2", target_bir_lowering=False, debug=False,
                   num_devices=NCORES)

    xt = nc.dram_tensor("xt", [DIM, T], bf16, kind="ExternalInput")
    wq = nc.dram_tensor("wq", [DIM, 256], bf16, kind="ExternalInput")
    wk = nc.dram_tensor("wk", [DIM, 256], bf16, kind="ExternalInput")
    wv = nc.dram_tensor("wv", [DIM, 256], bf16, kind="ExternalInput")
    wo = nc.dram_tensor("wo", [256, DIM], bf16, kind="ExternalInput")
    qkb = nc.dram_tensor("qkb", [128, 4], f32, kind="ExternalInput")
    z = nc.dram_tensor("z", [DIM, T], f32, kind="ExternalOutput")

    with tile.TileContext(nc) as tc:
        with (
            nc.allow_low_precision(
                reason="bf16 operands; PSUM accumulation stays fp32"),
            tc.tile_pool(name="scp", bufs=2, space="PSUM") as scp,
            tc.tile_pool(name="avp", bufs=2, space="PSUM") as avp,
            tc.tile_pool(name="flxA", bufs=1, space="PSUM") as flxA,
            tc.tile_pool(name="flxB", bufs=1, space="PSUM") as flxB,
            tc.tile_pool(name="consts", bufs=1) as consts,
            tc.tile_pool(name="kv", bufs=2) as kv,
            tc.tile_pool(name="xp", bufs=2 * NC) as xp,
            tc.tile_pool(name="work", bufs=4) as work,
            tc.tile_pool(name="outp", bufs=2 * NC) as outp,
        ):
            wq_sb = consts.tile([128, NDT, 256], bf16)
            wk_sb = consts.tile([128, NDT, 256], bf16)
            wv_sb = consts.tile([128, NDT, 256], bf16)
            wo_sb = consts.tile([128, 2, DIM], bf16)
            qkb_sb = consts.tile([128, 4], f32)
            ones_mm = consts.tile([1, 64], bf16)
            wu = consts.tile([128, CH], bf16, name="wu")

            # wk first (K-gen is the first real work), then the rest
            nc.gpsimd.dma_start(out=wk_sb, in_=wk.ap().rearrange("(dt p) m -> p dt m", p=128))
            nc.gpsimd.dma_start(out=qkb_sb, in_=qkb.ap())
            nc.gpsimd.dma_start(out=wv_sb, in_=wv.ap().rearrange("(dt p) m -> p dt m", p=128))
            nc.gpsimd.dma_start(out=wq_sb, in_=wq.ap().rearrange("(dt p) m -> p dt m", p=128))
            nc.gpsimd.dma_start(out=wo_sb, in_=wo.ap().rearrange("(pk p) n -> p pk n", p=128))

            xre = xt.ap().rearrange("(dt p) t -> p dt t", p=128)
            x_blks = {}
            for b in range(B):
                for c in range(NC):
                    tb = b * S + c * CH
                    xb = xp.tile([128, NDT, CH], bf16, tag="x", name=f"x{b}_{c}")
                    nc.sync.dma_start(out=xb, in_=xre[:, :, tb:tb + CH])
                    x_blks[(b, c)] = xb

            nc.vector.memset(wu, 0.25)
            nc.vector.memset(ones_mm, 1.0)

            kT, qT, va = {}, {}, {}
            for b in range(B):
                kT[b] = kv.tile([128, 2, S], bf16, tag="kT", name=f"kT{b}")
                qT[b] = kv.tile([128, 2, S], bf16, tag="qT", name=f"qT{b}")
                va[b] = kv.tile([128, NST, NH, 65], bf16, tag="va", name=f"va{b}")
                nc.vector.memset(va[b][:, :, :, 64:65], 1.0)

            # ---- startup warmup: ramp the PE clock while DMAs land ----
            for wi in range(NWARM):
                wp = flxB.tile([128, CH], f32, tag="fill", name=f"warm{wi}")
                nc.tensor.matmul(wp[:], wu[:, 0:128], wu[:], start=True, stop=True)

            # ---- fill queues + drain ----
            opq_m = []   # mandatory (QKV chains) — drained first, FIFO
            opq_f = []   # deferred (bc / wo-projection) — drained when m dry
            ndummy = [0]

            def dummy_op():
                wp = flxB.tile([128, CH], f32, tag="fill",
                               name=f"dum{ndummy[0]}")
                nc.tensor.matmul(wp[:], wu[:, 0:128], wu[:], start=True, stop=True)
                ndummy[0] += 1

            def drain(n):
                for _ in range(n):
                    if opq_m:
                        opq_m.pop(0)()
                    elif opq_f:
                        opq_f.pop(0)()
                    else:
                        dummy_op()

            def drain_all_m():
                while opq_m:
                    opq_m.pop(0)()

            # ---- QKV projection chain builders (list of closures) ----
            def qk_closures(b, c, wsb, dstT, bidx, pk, pool=None):
                st8 = {}
                pool_ = pool or flxA

                def part(d0):
                    def f():
                        if d0 == 0:
                            st8["ps"] = pool_.tile([128, CH], f32, tag="fill" if pool_ is flxB else "chain",
                                                   name=f"qk{b}_{c}_{bidx}_{pk}")
                        ps = st8["ps"]
                        for dt_i in (d0, d0 + 1):
                            nc.tensor.matmul(
                                ps[:], wsb[:, dt_i, 128 * pk:128 * pk + 128],
                                x_blks[(b, c)][:, dt_i, :],
                                start=(dt_i == 0), stop=(dt_i == NDT - 1))
                        if d0 == NDT - 2:
                            nc.vector.tensor_scalar(
                                dstT[b][:, pk, c * CH:(c + 1) * CH], ps[:],
                                qkb_sb[:, bidx + pk:bidx + pk + 1], None,
                                mybir.AluOpType.add)
                    return f

                return [part(d0) for d0 in range(0, NDT, 2)]

            def v_closures(b, c, tt, pool=None):
                st8 = {}
                pool_ = pool or flxA

                def part(d0):
                    def f():
                        if d0 == 0:
                            st8["ps"] = pool_.tile([128, CH], f32, tag="fill" if pool_ is flxB else "chain",
                                                   name=f"v{b}_{c}_{tt}")
                        ps = st8["ps"]
                        for dt_i in range(d0, d0 + 4):
                            nc.tensor.matmul(
                                ps[:, 0:256], x_blks[(b, c)][:, dt_i, 128 * tt:128 * tt + 128],
                                wv_sb[:, dt_i, :],
                                start=(dt_i == 0), stop=(dt_i == NDT - 1))
                        if d0 == NDT - 4:
                            st = c * 4 + tt
                            nc.vector.tensor_copy(
                                va[b][:, st, :, 0:64],
                                ps[:, 0:256].rearrange("p (h d) -> p h d", h=NH))
                    return f

                return [part(0), part(4)]

            def run_now(cls):
                for f in cls:
                    f()

            # ---- deferred output projection (wo' = lamf*wo@proj folded) ----
            outTs = {}

            def queue_bc(b, c, pk, u, rcp2m):
                outT = outTs[(b, c)]

                def f():
                    bc = flxB.tile([128, CH], f32, tag="fill",
                                   name=f"bc{b}_{c}_{pk}")
                    for hh in range(2):
                        nc.tensor.matmul(bc[64 * hh:64 * hh + 64, :],
                                         ones_mm[:], rcp2m[:, hh, :],
                                         start=True, stop=True)
                    nc.vector.tensor_mul(outT[:, pk, :], u[:], bc[:])
                opq_f.append(f)

            def queue_zt(b, c):
                tb = b * S + c * CH
                outT = outTs[(b, c)]

                def zt_op(eo):
                    pool_ = flxA if eo % 2 == 0 else flxB

                    def f():
                        zp = pool_.tile([128, CH], f32,
                                        tag="chain" if eo % 2 == 0 else "fill",
                                        name=f"zp{b}_{c}_{eo}")
                        for pk in range(2):
                            nc.tensor.matmul(
                                zp[:], wo_sb[:, pk, eo * 128:(eo + 1) * 128],
                                outT[:, pk, :],
                                start=(pk == 0), stop=(pk == 1))
                        zs = work.tile([128, CH], f32, tag="zs",
                                       name=f"zs{b}_{c}_{eo}")
                        nc.vector.tensor_copy(zs[:], zp[:])
                        nc.sync.dma_start(
                            out=z.ap()[eo * 128:(eo + 1) * 128, tb:tb + CH],
                            in_=zs[:])
                    return f

                for eo in range(NDT):
                    opq_f.append(zt_op(eo))

            # ---- attention block: one (batch, chunk, head-pair) ----
            def attn_block(b, c, pk):
                if pk == 0:
                    outTs[(b, c)] = outp.tile([128, 2, CH], bf16, tag="outT",
                                              name=f"outT{b}_{c}")
                avs = [avp.tile([128, CH], f32, tag="av",
                                name=f"av{b}_{c}_{pk}_{hh}") for hh in range(2)]

                def emit_av(hh, ex, sp):
                    h = 2 * pk + hh
                    for j in range(2):
                        st = 2 * sp + j
                        nc.tensor.matmul(
                            avs[hh][0:65, :], va[b][:, st, h, :], ex[:, j, :],
                            start=(st == 0), stop=(st == NST - 1))

                prev_ex = None
                for sp in range(NST // 2):
                    exs = []
                    for hh in range(2):
                        row = 64 * hh
                        sc = scp.tile([128, 2, CH], f32, tag="sc",
                                      name=f"sc{b}_{c}_{pk}_{sp}_{hh}")
                        for j in range(2):
                            st = 2 * sp + j
                            nc.tensor.matmul(
                                sc[:, j, :],
                                kT[b][row:row + 64, pk, st * 128:(st + 1) * 128],
                                qT[b][row:row + 64, pk, c * CH:(c + 1) * CH],
                                start=True, stop=True)
                        ex = work.tile([128, 2, CH], bf16, tag="ex", bufs=4,
                                       name=f"ex{b}_{c}_{pk}_{sp}_{hh}")
                        nc.scalar.activation(
                            ex[:], sc[:], mybir.ActivationFunctionType.Exp)
                        exs.append(ex)
                    drain(1)
                    if prev_ex is not None:
                        emit_av(1, prev_ex[1], sp - 1)
                    else:
                        drain(1)
                    drain(1)
                    emit_av(0, exs[0], sp)
                    prev_ex = exs
                drain(1)
                emit_av(1, prev_ex[1], NST // 2 - 1)

                # normalization epilogue (DVE; bc matmuls deferred to fill queue)
                den2 = work.tile([1, 2, CH], f32, tag="den", bufs=2,
                                 name=f"den{b}_{c}_{pk}")
                nc.vector.tensor_copy(den2[:, 0, :], avs[0][64:65, :])
                nc.vector.tensor_copy(den2[:, 1, :], avs[1][64:65, :])
                rcp2 = work.tile([1, 2, CH], f32, tag="rcp", bufs=2,
                                 name=f"rcp{b}_{c}_{pk}")
                nc.vector.reciprocal_approx_fast(rcp2[:], den2[:])
                rcp2m = work.tile([1, 2, CH], bf16, tag="rcpm", bufs=4,
                                  name=f"rcpm{b}_{c}_{pk}")
                nc.vector.tensor_copy(rcp2m[:], rcp2[:])
                u = work.tile([128, CH], bf16, tag="u", bufs=4,
                              name=f"u{b}_{c}_{pk}")
                nc.vector.tensor_copy(u[0:64, :], avs[0][0:64, :])
                nc.vector.tensor_copy(u[64:128, :], avs[1][0:64, :])
                queue_bc(b, c, pk, u, rcp2m)
                if pk == 1:
                    queue_zt(b, c)

            # ================= program =================
            # phase A(b0) inline & dense: K all chunks, V all chunks, Q(c0).
            # Chains alternate the two flex PSUM banks so chain n+1's matmuls
            # never wait on chain n's PSUM->SBUF copy (keeps the PE gap-free).
            par = [0]

            def run_alt(mk):
                run_now(mk([flxA, flxB][par[0] & 1]))
                par[0] += 1

            for c in range(NC):
                for pk in range(2):
                    run_alt(lambda p, c=c, pk=pk: qk_closures(0, c, wk_sb, kT, 2, pk, p))
            for c in range(NC):
                for tt in range(4):
                    run_alt(lambda p, c=c, tt=tt: v_closures(0, c, tt, p))
            for pk in range(2):
                run_alt(lambda p, pk=pk: qk_closures(0, 0, wq_sb, qT, 0, pk, p))

            # queue the rest: Q(b0,c1..), then all of batch 1's projections
            for c in range(1, NC):
                for pk in range(2):
                    opq_m += qk_closures(0, c, wq_sb, qT, 0, pk)
            for c in range(NC):
                for pk in range(2):
                    opq_m += qk_closures(1, c, wk_sb, kT, 2, pk)
            for pk in range(2):
                opq_m += qk_closures(1, 0, wq_sb, qT, 0, pk)
            for c in range(NC):
                for tt in range(4):
                    opq_m += v_closures(1, c, tt)
            for c in range(1, NC):
                for pk in range(2):
                    opq_m += qk_closures(1, c, wq_sb, qT, 0, pk)

            # phase B(b0): fill slots consume opq_m (A(b1) work)
            for c in range(NC):
                for pk in range(2):
                    attn_block(0, c, pk)

            # everything batch 1 needs must be emitted before its blocks
            drain_all_m()

            # phase B(b1): fill slots consume opq_f (deferred wo-projections)
            for c in range(NC):
                for pk in range(2):
                    attn_block(1, c, pk)

            while opq_f:
                opq_f.pop(0)()

    nc.compile()
    return nc


def get_lambda(lambda_param, layer_idx):
    lf = np.clip(float(np.asarray(layer_idx)) * 0.3, 0.0, 5.0)
    offset = 0.6 * np.exp(-lf)
    lam = (1.0 / (1.0 + np.exp(-float(np.asarray(lambda_param).reshape(-1)[0])))
           ) * (1.0 - offset) + 0.2
    return float(np.clip(lam, 0.1, 0.9))


def prep(inputs, S=2048):
    """Host-side shard prep: returns (in_maps, bias_vec)."""
    x = np.asarray(inputs["x"], np.float32)
    T = B * S
    x2 = np.ascontiguousarray(x.reshape(T, DIM))
    xt = np.ascontiguousarray(x2.T).astype(MM_NP)

    lam = get_lambda(inputs["lambda_param"], inputs["layer_idx"])
    pw = np.asarray(inputs["proj_w"], np.float32)
    sc_q = 1.0 / np.sqrt(HD)

    in_maps = []
    for core in range(NCORES):
        br = core // 4 + 1
        lamf = (1.0 - lam) if br == 1 else lam
        hs = slice(4 * (core % 4), 4 * (core % 4) + 4)

        def pick(w, scale=1.0):
            wa = np.asarray(w, np.float32)[:, hs] * scale
            return np.ascontiguousarray(wa.reshape(DIM, NH * HD)).astype(MM_NP)

        # per-partition q/k biases: row r of the pk head-pair tile is
        # head (2pk + r//64), dim r%64
        bq = np.asarray(inputs[f"bq{br}"], np.float32)[hs] * sc_q
        bk = np.asarray(inputs[f"bk{br}"], np.float32)[hs]
        qkb = np.zeros((128, 4), np.float32)
        for pk in range(2):
            qkb[:, 0 + pk] = bq[2 * pk:2 * pk + 2].reshape(128)
            qkb[:, 2 + pk] = bk[2 * pk:2 * pk + 2].reshape(128)

        wo_c = np.ascontiguousarray(
            ((np.asarray(inputs[f"wo{br}"], np.float32)[hs] * lamf
              ).reshape(256, DIM) @ pw).astype(MM_NP))
        in_maps.append({
            "xt": xt,
            "wq": pick(inputs[f"wq{br}"], sc_q),
            "wk": pick(inputs[f"wk{br}"]),
            "wv": pick(inputs[f"wv{br}"]),
            "wo": wo_c,
            "qkb": qkb,
        })

    lam64 = np.float64(lam)
    bias_vec = np.zeros((DIM,), np.float64)
    for br, lamf in ((1, 1.0 - lam64), (2, lam64)):
        bo = np.asarray(inputs[f"bo{br}"], np.float64)
        bv = np.asarray(inputs[f"bv{br}"], np.float64).reshape(H * HD)
        wo_full = np.asarray(inputs[f"wo{br}"], np.float64).reshape(H * HD, DIM)
        bias_vec += lamf * (bo + bv @ wo_full)
    bias_vec = bias_vec @ pw.astype(np.float64) \
        + np.asarray(inputs["proj_b"], np.float64)
    return in_maps, bias_vec


_NC_CACHE = {}


def _get_nc(S=2048):
    if S not in _NC_CACHE:
        _NC_CACHE[S] = build(S)
    return _NC_CACHE[S]


def run(inputs, S=2048, trace=False):
    """Returns (full_output, exec_time_ns_or_None)."""
    from concourse import bass_utils

    nc = _get_nc(S)
    in_maps, bias_vec = prep(inputs, S)
    res = bass_utils.run_bass_kernel_spmd(
        nc, in_maps, core_ids=list(range(NCORES)), trace=trace)
    accT = np.zeros((DIM, B * S), np.float64)
    for c in range(NCORES):
        accT += res.results[c]["z"].astype(np.float64)
    out = (accT.T + bias_vec).reshape(B, S, DIM).astype(np.float32)
    return out, res.exec_time_ns


def kernel(**inputs):
    out, _ = run(inputs, S=2048, trace=False)
    return out
